# revision 5
# baseline (speedup 1.0000x reference)
"""Trainium2 Bass kernel for nn_AttentionLayer (B=32, Sx=Sy=2048, D=1024).

reference:
    S   = einsum('byd,bxd->byx', y, x) / sqrt(D)       # [B, Sy, Sx]
    W   = softmax(S, axis=2)
    visual = mean(S, axis=1)                           # [B, Sx]
    out    = mean(W @ x, axis=1)                       # [B, D]

Key algebra: both outputs are means over the Sy axis, so
    visual = (mean_y y) @ x^T / sqrt(D)                 (tiny GEMV)
    out    = cbar @ x,  cbar[x] = mean_y W[y, x]        (tiny GEMV)
Only the scores matmul + softmax colsum are heavy. The second big matmul
(W @ x, equal FLOPs to the first) is eliminated entirely.

Sharding: batch dim across 8 cores, 4 batches per core (pure data parallel).
"""

import contextlib
import numpy as np

import concourse.bass as bass
import concourse.bacc as bacc
import concourse.tile as tile
import concourse.mybir as mybir
from concourse.bass_utils import run_bass_kernel_spmd
from concourse.bass_interp import get_hw_module
from concourse.masks import make_identity

B, S, D = 32, 2048, 1024
NCORES = 8
PB = B // NCORES          # batches per core = 4
NM = S // 128             # 16 row tiles (y)
NK = D // 128             # 8 contraction tiles (d)
NCH = S // 512            # 4 column chunks of 512 (x)
SCALE = 1.0 / 32.0        # 1/sqrt(D)

F32 = mybir.dt.float32
F32R = mybir.dt.float32r
AF = mybir.ActivationFunctionType
ALU = mybir.AluOpType
AX = mybir.AxisListType

# dtype knobs for the PE stages (F32R = full-speed reduced precision)
MM_DT = F32R       # scores matmul
CMM_DT = F32R      # softmax-colsum matmul
VMM_DT = F32R      # visual matmul


def _emit_batch(nc, tc, b, aps, pools, ident):
    xin, yin, visual, aout = aps
    (p_xT, p_xnat, p_ynat, p_yTm, p_E, p_small,
     ps_pool, pt_pool, pc_pool) = pools

    # ---------------- stage B: load x, transpose to xT[d, k, x] ----------
    # xT[p, k, i*128+j] = x[b, i*128+j, k*128+p]
    xT = p_xT.tile([128, NK, S], F32R, tag="xT")
    for i in range(NM):
        xn = p_xnat.tile([128, D], F32, tag="xnat")
        nc.sync.dma_start(xn[:], xin[b, i * 128:(i + 1) * 128, :])
        for h in range(2):
            pt = pt_pool.tile([128, 512], F32, tag="pt")
            for j in range(4):
                k = h * 4 + j
                nc.tensor.transpose(
                    pt[:, j * 128:(j + 1) * 128],
                    xn[:, k * 128:(k + 1) * 128],
                    ident[:],
                )
            dst = xT[:, h * 4:(h + 1) * 4, i * 128:(i + 1) * 128]
            src = pt[:].rearrange("p (a c) -> p a c", a=4)
            if (2 * i + h) % 2 == 0:
                nc.scalar.activation(dst, src, AF.Copy)
            else:
                nc.vector.tensor_copy(dst, src)

    # ---------------- stage C: scores + softmax + colsum ------------------
    ybar = p_small.tile([128, NK], F32, tag="ybar")
    psum_c = pc_pool.tile([128, 4 * 512], F32, tag="pc")

    def y_transp(m):
        """load y row-tile m, transpose into yt[p, k*128+j] = y[m*128+j, k*128+p]"""
        yn = p_ynat.tile([128, D], F32, tag="ynat")
        nc.sync.dma_start(yn[:], yin[b, m * 128:(m + 1) * 128, :])
        yt = p_yTm.tile([128, D], F32R, tag="yTm")
        for h in range(2):
            pt = pt_pool.tile([128, 512], F32, tag="pt")
            for j in range(4):
                k = h * 4 + j
                nc.tensor.transpose(
                    pt[:, j * 128:(j + 1) * 128],
                    yn[:, k * 128:(k + 1) * 128],
                    ident[:],
                )
            if h == 0:
                nc.scalar.activation(yt[:, h * 512:(h + 1) * 512], pt[:], AF.Copy)
            else:
                nc.vector.tensor_copy(yt[:, h * 512:(h + 1) * 512], pt[:])
        return yt

    def scores_chunk(yt, Em, rsum, n):
        ps = ps_pool.tile([128, 512], F32, tag="ps")
        for k in range(NK):
            nc.tensor.matmul(
                ps[:],
                lhsT=yt[:, k * 128:(k + 1) * 128],
                rhs=xT[:, k, n * 512:(n + 1) * 512],
                start=(k == 0),
                stop=(k == NK - 1),
            )
        nc.scalar.activation(
            Em[:, n * 512:(n + 1) * 512], ps[:], AF.Exp,
            scale=SCALE, accum_out=rsum[:, n:n + 1],
        )

    def c_mms(m, Em, recip):
        for n in range(NCH):
            nc.tensor.matmul(
                psum_c[0:1, n * 512:(n + 1) * 512],
                lhsT=recip[:],
                rhs=Em[:, n * 512:(n + 1) * 512],
                start=(m == 0),
                stop=(m == NM - 1),
            )

    yt = y_transp(0)
    prev = None  # (m, Em, recip) pending softmax-colsum matmuls
    for m in range(NM):
        # ybar accumulation on DVE (sum of y rows, in transposed layout)
        ytv = yt[:].bitcast(F32).rearrange("p (k j) -> p k j", k=NK)
        if m == 0:
            nc.vector.reduce_sum(ybar[:], ytv, axis=AX.X)
        else:
            ybs = p_small.tile([128, NK], F32, tag="ybs")
            nc.vector.reduce_sum(ybs[:], ytv, axis=AX.X)
            nc.vector.tensor_add(ybar[:], ybar[:], ybs[:])

        Em = p_E.tile([128, S], F32R, tag="E")
        rsum = p_small.tile([128, NCH], F32, tag="rsum")
        scores_chunk(yt, Em, rsum, 0)
        scores_chunk(yt, Em, rsum, 1)
        if prev is not None:
            c_mms(*prev)
        scores_chunk(yt, Em, rsum, 2)
        scores_chunk(yt, Em, rsum, 3)
        if m + 1 < NM:
            yt = y_transp(m + 1)
        # rowsum total + reciprocal (DVE)
        rtot = p_small.tile([128, 1], F32, tag="rtot")
        nc.vector.reduce_sum(rtot[:], rsum[:], axis=AX.X)
        recip = p_small.tile([128, 1], F32R, tag="recip")
        with nc.allow_low_precision(reason="softmax recip rounded to f32r for PE"):
            nc.vector.reciprocal(recip[:], rtot[:])
        prev = (m, Em, recip)
    c_mms(*prev)

    # ---------------- stage D: outputs ------------------------------------
    # cbar = colmean of softmax weights, in [1, S] on partition 0
    csb = p_small.tile([1, S], F32, tag="csb")
    nc.scalar.activation(csb[0:1, :], psum_c[0:1, :], AF.Copy, scale=1.0 / S)
    cbb = p_E.tile([128, S], F32, tag="E")        # broadcast of cbar
    nc.gpsimd.partition_broadcast(cbb[:], csb[0:1, :])

    # out[d] = sum_x cbar[x] * xT[d, x]  (DVE reduce along free dim)
    scratch = p_E.tile([128, S], F32, tag="E")
    aout_sb = p_small.tile([128, NK], F32, tag="aout_sb")
    for k in range(NK):
        nc.vector.scalar_tensor_tensor(
            out=scratch[:],
            in0=xT[:, k, :].bitcast(F32),
            scalar=1.0,
            in1=cbb[:],
            op0=ALU.mult,
            op1=ALU.mult,
            accum_out=aout_sb[:, k:k + 1],
        )

    # visual = ybar_scaled @ xT  (PE GEMV, accumulate over k)
    ybar_r = p_small.tile([128, NK], F32R, tag="ybar_r")
    nc.vector.tensor_scalar_mul(ybar_r[:], ybar[:], SCALE / S)
    vis_sb = p_small.tile([1, S], F32, tag="vis_sb")
    for n in range(NCH):
        pv = ps_pool.tile([128, 512], F32, tag="ps")
        for k in range(NK):
            nc.tensor.matmul(
                pv[0:1, :],
                lhsT=ybar_r[:, k:k + 1],
                rhs=xT[:, k, n * 512:(n + 1) * 512],
                start=(k == 0),
                stop=(k == NK - 1),
            )
        nc.scalar.activation(vis_sb[0:1, n * 512:(n + 1) * 512], pv[0:1, :], AF.Copy)
    nc.sync.dma_start(visual[b:b + 1, :], vis_sb[0:1, :])

    # aout [128, NK] -> transpose to [NK, 128] so DRAM write is contiguous
    ptt = pt_pool.tile([128, 512], F32, tag="pt")
    nc.tensor.transpose(ptt[0:NK, 0:128], aout_sb[:], ident[:])
    aout_t = p_small.tile([NK, 128], F32, tag="aout_t")
    nc.scalar.activation(aout_t[0:NK, :], ptt[0:NK, 0:128], AF.Copy)
    nc.sync.dma_start(
        aout[b:b + 1, :].rearrange("o (a c) -> (o a) c", a=NK),
        aout_t[0:NK, :],
    )


def build():
    nc = bacc.Bacc("TRN2", target_bir_lowering=False, debug=False,
                   num_devices=NCORES)
    xin = nc.dram_tensor("xin", [PB, S, D], F32, kind="ExternalInput").ap()
    yin = nc.dram_tensor("yin", [PB, S, D], F32, kind="ExternalInput").ap()
    visual = nc.dram_tensor("visual", [PB, S], F32, kind="ExternalOutput").ap()
    aout = nc.dram_tensor("aout", [PB, D], F32, kind="ExternalOutput").ap()
    aps = (xin, yin, visual, aout)

    with contextlib.ExitStack() as ctx:
        tc = ctx.enter_context(tile.TileContext(nc))
        p_xT = ctx.enter_context(tc.tile_pool(name="xT", bufs=1))
        p_xnat = ctx.enter_context(tc.tile_pool(name="xnat", bufs=16))
        p_ynat = ctx.enter_context(tc.tile_pool(name="ynat", bufs=4))
        p_yTm = ctx.enter_context(tc.tile_pool(name="yTm", bufs=3))
        p_E = ctx.enter_context(tc.tile_pool(name="E", bufs=2))
        p_small = ctx.enter_context(tc.tile_pool(name="small", bufs=2))
        p_const = ctx.enter_context(tc.tile_pool(name="const", bufs=1))
        ps_pool = ctx.enter_context(tc.tile_pool(name="ps", bufs=2, space="PSUM"))
        pt_pool = ctx.enter_context(tc.tile_pool(name="pt", bufs=2, space="PSUM"))
        pc_pool = ctx.enter_context(tc.tile_pool(name="pc", bufs=1, space="PSUM"))
        pools = (p_xT, p_xnat, p_ynat, p_yTm, p_E, p_small,
                 ps_pool, pt_pool, pc_pool)

        ident = p_const.tile([128, 128], F32, tag="ident")
        make_identity(nc, ident[:])

        for b in range(PB):
            _emit_batch(nc, tc, b, aps, pools, ident)

    nc.compile()
    nc.m = get_hw_module(nc.m)
    return nc


_NC_CACHE = None


def _get_nc():
    global _NC_CACHE
    if _NC_CACHE is None:
        _NC_CACHE = build()
    return _NC_CACHE


def _run(input_x, input_y, trace=False, **kw):
    nc = _get_nc()
    input_x = np.ascontiguousarray(np.asarray(input_x, dtype=np.float32))
    input_y = np.ascontiguousarray(np.asarray(input_y, dtype=np.float32))
    in_maps = [
        {"xin": input_x[c * PB:(c + 1) * PB], "yin": input_y[c * PB:(c + 1) * PB]}
        for c in range(NCORES)
    ]
    res = run_bass_kernel_spmd(nc, in_maps, core_ids=list(range(NCORES)),
                               trace=trace, **kw)
    visual = np.concatenate([res.results[c]["visual"] for c in range(NCORES)], axis=0)
    aout = np.concatenate([res.results[c]["aout"] for c in range(NCORES)], axis=0)
    return (visual, aout), res


def kernel(input_x, input_y):
    (visual, aout), _ = _run(input_x, input_y)
    return visual, aout


# revision 7
# speedup vs baseline: 1.0986x; 1.0986x over previous
"""Trainium2 Bass kernel for nn_AttentionLayer (B=32, Sx=Sy=2048, D=1024).

reference:
    S   = einsum('byd,bxd->byx', y, x) / sqrt(D)       # [B, Sy, Sx]
    W   = softmax(S, axis=2)
    visual = mean(S, axis=1)                           # [B, Sx]
    out    = mean(W @ x, axis=1)                       # [B, D]

Key algebra: both outputs are means over the Sy axis, so
    visual = (mean_y y) @ x^T / sqrt(D)                 (tiny GEMV)
    out    = cbar @ x,  cbar[x] = mean_y W[y, x]        (tiny GEMV)
Only the scores matmul + softmax colsum are heavy. The second big matmul
(W @ x, equal FLOPs to the first) is eliminated entirely.

Sharding: batch dim across 8 cores, 4 batches per core (pure data parallel).
"""

import contextlib
import numpy as np

import concourse.bass as bass
import concourse.bacc as bacc
import concourse.tile as tile
import concourse.mybir as mybir
from concourse.bass_utils import run_bass_kernel_spmd
from concourse.bass_interp import get_hw_module
from concourse.masks import make_identity

B, S, D = 32, 2048, 1024
NCORES = 8
PB = B // NCORES          # batches per core = 4
NM = S // 128             # 16 row tiles (y)
NK = D // 128             # 8 contraction tiles (d)
NCH = S // 512            # 4 column chunks of 512 (x)
SCALE = 1.0 / 32.0        # 1/sqrt(D)

F32 = mybir.dt.float32
F32R = mybir.dt.float32r
BF16 = mybir.dt.bfloat16
AF = mybir.ActivationFunctionType
ALU = mybir.AluOpType
AX = mybir.AxisListType

TRANSPOSE_F32R = True     # PE transposes at 1.5 cyc/row instead of 2.0


def _emit_batch(nc, tc, b, aps, pools, ident, identr, pending_tail):
    """Emit one batch. Returns a closure emitting this batch's deferred
    aout drain (transpose+copy+DMA), to be emitted inside the NEXT batch so
    the PE doesn't stall on the DVE out-GEMV at the batch boundary."""
    xin, yin, visual, aout = aps
    (p_xT, p_xTb, p_xnat, p_ynat, p_yTm, p_E, p_cb, p_small,
     ps_pool, pc_pool) = pools

    tdt = F32R if TRANSPOSE_F32R else F32
    tident = identr if TRANSPOSE_F32R else ident[:]

    def transpose_8(src_tile):
        """8 PE transposes of one [128, 1024] natural tile into 2 psum tiles;
        returns the two [128, 512] psum tiles (k-chunks 0-3 and 4-7)."""
        pts = []
        for h in range(2):
            pt = ps_pool.tile([128, 512], tdt, tag="ps")
            for j in range(4):
                k = h * 4 + j
                nc.tensor.transpose(
                    pt[:, j * 128:(j + 1) * 128],
                    src_tile[:, k * 128:(k + 1) * 128],
                    tident,
                )
            pts.append(pt)
        return pts

    # ---------------- stage B: load x, transpose to xT[d, k, x] ----------
    # xT[p, k, i*128+j] = x[b, i*128+j, k*128+p]; xTb = bf16 shadow for the
    # final out-GEMV (so it doesn't WAR-block next batch's xT rebuild).
    xT = p_xT.tile([128, NK, S], F32R, tag="xT")
    xTb = p_xTb.tile([128, NK, S], BF16, tag="xTb")
    for i in range(NM):
        xn = p_xnat.tile([128, D], tdt, tag="xnat")
        nc.sync.dma_start(xn[:], xin[b, i * 128:(i + 1) * 128, :].bitcast(tdt))
        pts = transpose_8(xn)
        for h in range(2):
            dst = xT[:, h * 4:(h + 1) * 4, i * 128:(i + 1) * 128]
            dstb = xTb[:, h * 4:(h + 1) * 4, i * 128:(i + 1) * 128]
            src = pts[h][:].bitcast(F32).rearrange("p (a c) -> p a c", a=4)
            if h == 0:
                nc.scalar.activation(dst, src, AF.Copy)
                nc.vector.tensor_copy(dstb, src)
            else:
                nc.vector.tensor_copy(dst, src)
                nc.scalar.activation(dstb, src, AF.Copy)
        if i == 3 and pending_tail is not None:
            pending_tail()

    # ---------------- stage C: scores + softmax + colsum ------------------
    ybar = p_small.tile([128, NK], F32, tag="ybar")
    psum_c = pc_pool.tile([128, 4 * 512], F32, tag="pc")

    def y_transp(m):
        """load y row-tile m, transpose into yt[p, k*128+j] = y[m*128+j, k*128+p]"""
        yn = p_ynat.tile([128, D], tdt, tag="ynat")
        nc.sync.dma_start(yn[:], yin[b, m * 128:(m + 1) * 128, :].bitcast(tdt))
        yt = p_yTm.tile([128, D], F32R, tag="yTm")
        pts = transpose_8(yn)
        for h in range(2):
            src = pts[h][:].bitcast(F32)
            if h == 0:
                nc.scalar.activation(yt[:, h * 512:(h + 1) * 512], src, AF.Copy)
            else:
                nc.vector.tensor_copy(yt[:, h * 512:(h + 1) * 512], src)
        return yt

    def scores_chunk(yt, Em, rsum, n):
        ps = ps_pool.tile([128, 512], F32, tag="ps")
        for k in range(NK):
            nc.tensor.matmul(
                ps[:],
                lhsT=yt[:, k * 128:(k + 1) * 128],
                rhs=xT[:, k, n * 512:(n + 1) * 512],
                start=(k == 0),
                stop=(k == NK - 1),
            )
        nc.scalar.activation(
            Em[:, n * 512:(n + 1) * 512], ps[:], AF.Exp,
            scale=SCALE, accum_out=rsum[:, n:n + 1],
        )

    def c_mms(m, Em, recip):
        for n in range(NCH):
            nc.tensor.matmul(
                psum_c[0:1, n * 512:(n + 1) * 512],
                lhsT=recip[:],
                rhs=Em[:, n * 512:(n + 1) * 512],
                start=(m == 0),
                stop=(m == NM - 1),
            )

    yt = y_transp(0)
    prev = None  # (m, Em, recip) pending softmax-colsum matmuls
    for m in range(NM):
        # ybar accumulation on DVE (sum of y rows, in transposed layout)
        ytv = yt[:].bitcast(F32).rearrange("p (k j) -> p k j", k=NK)
        if m == 0:
            nc.vector.reduce_sum(ybar[:], ytv, axis=AX.X)
        else:
            ybs = p_small.tile([128, NK], F32, tag="ybs")
            nc.vector.reduce_sum(ybs[:], ytv, axis=AX.X)
            nc.vector.tensor_add(ybar[:], ybar[:], ybs[:])

        Em = p_E.tile([128, S], F32R, tag="E")
        rsum = p_small.tile([128, NCH], F32, tag="rsum")
        scores_chunk(yt, Em, rsum, 0)
        scores_chunk(yt, Em, rsum, 1)
        yt_next = y_transp(m + 1) if m + 1 < NM else None
        if prev is not None:
            c_mms(*prev)
        scores_chunk(yt, Em, rsum, 2)
        scores_chunk(yt, Em, rsum, 3)
        # rowsum total + reciprocal (DVE)
        rtot = p_small.tile([128, 1], F32, tag="rtot")
        nc.vector.reduce_sum(rtot[:], rsum[:], axis=AX.X)
        recip = p_small.tile([128, 1], F32R, tag="recip")
        with nc.allow_low_precision(reason="softmax recip rounded to f32r for PE"):
            nc.vector.reciprocal(recip[:], rtot[:])
        prev = (m, Em, recip)
        yt = yt_next
    c_mms(*prev)

    # ---------------- stage D: outputs ------------------------------------
    # visual = ybar_scaled @ xT  (PE GEMV, accumulate over k) — independent
    # of the softmax tail, emit first so PE flows straight into it.
    ybar_r = p_small.tile([128, NK], F32R, tag="ybar_r")
    nc.vector.tensor_scalar_mul(ybar_r[:], ybar[:], SCALE / S)
    vis_sb = p_small.tile([1, S], F32, tag="vis_sb")
    for n in range(NCH):
        pv = ps_pool.tile([128, 512], F32, tag="ps")
        for k in range(NK):
            nc.tensor.matmul(
                pv[0:1, :],
                lhsT=ybar_r[:, k:k + 1],
                rhs=xT[:, k, n * 512:(n + 1) * 512],
                start=(k == 0),
                stop=(k == NK - 1),
            )
        nc.scalar.activation(vis_sb[0:1, n * 512:(n + 1) * 512], pv[0:1, :], AF.Copy)
    nc.sync.dma_start(visual[b:b + 1, :], vis_sb[0:1, :])

    # cbar = colmean of softmax weights, bf16, broadcast to all partitions
    csb = p_cb.tile([1, S], BF16, tag="csb")
    with nc.allow_low_precision(reason="cbar rounded to bf16 for the out GEMV"):
        nc.scalar.activation(csb[0:1, :], psum_c[0:1, :], AF.Copy, scale=1.0 / S)
    cbb = p_cb.tile([128, S], BF16, tag="cbb")
    nc.gpsimd.partition_broadcast(cbb[:], csb[0:1, :])

    # out[d] = sum_x cbar[x] * x[x, d] via DVE multiply+accumulate over the
    # bf16 shadow; writes in-place into xTb (dead after this).
    aout_sb = p_small.tile([128, NK], F32, tag="aout_sb")
    for k in range(NK):
        nc.vector.scalar_tensor_tensor(
            out=xTb[:, k, :],
            in0=xTb[:, k, :],
            scalar=1.0,
            in1=cbb[:],
            op0=ALU.mult,
            op1=ALU.mult,
            accum_out=aout_sb[:, k:k + 1],
        )

    def tail():
        # aout [128, NK] -> transpose to [NK, 128] so DRAM write is contiguous
        ptt = ps_pool.tile([128, 512], F32, tag="ps")
        nc.tensor.transpose(ptt[0:NK, 0:128], aout_sb[:], ident[:])
        aout_t = p_small.tile([NK, 128], F32, tag="aout_t")
        nc.scalar.activation(aout_t[0:NK, :], ptt[0:NK, 0:128], AF.Copy)
        nc.sync.dma_start(
            aout[b:b + 1, :].rearrange("o (a c) -> (o a) c", a=NK),
            aout_t[0:NK, :],
        )
    return tail


def build():
    nc = bacc.Bacc("TRN2", target_bir_lowering=False, debug=False,
                   num_devices=NCORES)
    xin = nc.dram_tensor("xin", [PB, S, D], F32, kind="ExternalInput").ap()
    yin = nc.dram_tensor("yin", [PB, S, D], F32, kind="ExternalInput").ap()
    visual = nc.dram_tensor("visual", [PB, S], F32, kind="ExternalOutput").ap()
    aout = nc.dram_tensor("aout", [PB, D], F32, kind="ExternalOutput").ap()
    aps = (xin, yin, visual, aout)

    with contextlib.ExitStack() as ctx:
        tc = ctx.enter_context(tile.TileContext(nc))
        p_xT = ctx.enter_context(tc.tile_pool(name="xT", bufs=1))
        p_xTb = ctx.enter_context(tc.tile_pool(name="xTb", bufs=1))
        p_xnat = ctx.enter_context(tc.tile_pool(name="xnat", bufs=10))
        p_ynat = ctx.enter_context(tc.tile_pool(name="ynat", bufs=3))
        p_yTm = ctx.enter_context(tc.tile_pool(name="yTm", bufs=2))
        p_E = ctx.enter_context(tc.tile_pool(name="E", bufs=2))
        p_cb = ctx.enter_context(tc.tile_pool(name="cb", bufs=1))
        p_small = ctx.enter_context(tc.tile_pool(name="small", bufs=2))
        p_const = ctx.enter_context(tc.tile_pool(name="const", bufs=1))
        ps_pool = ctx.enter_context(tc.tile_pool(name="ps", bufs=4, space="PSUM"))
        pc_pool = ctx.enter_context(tc.tile_pool(name="pc", bufs=1, space="PSUM"))
        pools = (p_xT, p_xTb, p_xnat, p_ynat, p_yTm, p_E, p_cb, p_small,
                 ps_pool, pc_pool)

        ident = p_const.tile([128, 128], F32, tag="ident")
        make_identity(nc, ident[:])
        identr_t = p_const.tile([128, 128], F32R, tag="identr")
        nc.scalar.activation(identr_t[:], ident[:], AF.Copy)
        identr = identr_t[:]

        tail = None
        for b in range(PB):
            tail = _emit_batch(nc, tc, b, aps, pools, ident, identr, tail)
        tail()

    nc.compile()
    nc.m = get_hw_module(nc.m)
    return nc


_NC_CACHE = None


def _get_nc():
    global _NC_CACHE
    if _NC_CACHE is None:
        _NC_CACHE = build()
    return _NC_CACHE


def _run(input_x, input_y, trace=False, **kw):
    nc = _get_nc()
    input_x = np.ascontiguousarray(np.asarray(input_x, dtype=np.float32))
    input_y = np.ascontiguousarray(np.asarray(input_y, dtype=np.float32))
    in_maps = [
        {"xin": input_x[c * PB:(c + 1) * PB], "yin": input_y[c * PB:(c + 1) * PB]}
        for c in range(NCORES)
    ]
    res = run_bass_kernel_spmd(nc, in_maps, core_ids=list(range(NCORES)),
                               trace=trace, **kw)
    visual = np.concatenate([res.results[c]["visual"] for c in range(NCORES)], axis=0)
    aout = np.concatenate([res.results[c]["aout"] for c in range(NCORES)], axis=0)
    return (visual, aout), res


def kernel(input_x, input_y):
    (visual, aout), _ = _run(input_x, input_y)
    return visual, aout


# revision 8
# speedup vs baseline: 1.1577x; 1.0538x over previous
"""Trainium2 Bass kernel for nn_AttentionLayer (B=32, Sx=Sy=2048, D=1024).

reference:
    S   = einsum('byd,bxd->byx', y, x) / sqrt(D)       # [B, Sy, Sx]
    W   = softmax(S, axis=2)
    visual = mean(S, axis=1)                           # [B, Sx]
    out    = mean(W @ x, axis=1)                       # [B, D]

Key algebra: both outputs are means over the Sy axis, so
    visual = (mean_y y) @ x^T / sqrt(D)                 (tiny GEMV)
    out    = cbar @ x,  cbar[x] = mean_y W[y, x]        (tiny GEMV)
Only the scores matmul + softmax colsum are heavy. The second big matmul
(W @ x, equal FLOPs to the first) is eliminated entirely.

Sharding: batch dim across 8 cores, 4 batches per core (pure data parallel).
"""

import contextlib
import numpy as np

import concourse.bass as bass
import concourse.bacc as bacc
import concourse.tile as tile
import concourse.mybir as mybir
from concourse.bass_utils import run_bass_kernel_spmd
from concourse.bass_interp import get_hw_module
from concourse.masks import make_identity

B, S, D = 32, 2048, 1024
NCORES = 8
PB = B // NCORES          # batches per core = 4
NM = S // 128             # 16 row tiles (y)
NK = D // 128             # 8 contraction tiles (d)
NCH = S // 512            # 4 column chunks of 512 (x)
SCALE = 1.0 / 32.0        # 1/sqrt(D)

F32 = mybir.dt.float32
F32R = mybir.dt.float32r
BF16 = mybir.dt.bfloat16
AF = mybir.ActivationFunctionType
ALU = mybir.AluOpType
AX = mybir.AxisListType

TRANSPOSE_F32R = True     # PE transposes at 1.5 cyc/row instead of 2.0


def _emit_batch(nc, tc, b, aps, pools, ident, identr, pending_tail):
    """Emit one batch. Returns a closure emitting this batch's deferred
    aout drain (transpose+copy+DMA), to be emitted inside the NEXT batch so
    the PE doesn't stall on the DVE out-GEMV at the batch boundary."""
    xin, yin, visual, aout = aps
    (p_xT, p_xTb, p_xnat, p_ynat, p_yTm, p_E, p_cb, p_small,
     ps_pool, pc_pool) = pools

    tdt = F32R if TRANSPOSE_F32R else F32
    tident = identr if TRANSPOSE_F32R else ident[:]

    def transpose_8(src_tile):
        """8 PE transposes of one [128, 1024] natural tile into 2 psum tiles;
        returns the two [128, 512] psum tiles (k-chunks 0-3 and 4-7)."""
        pts = []
        for h in range(2):
            pt = ps_pool.tile([128, 512], tdt, tag="ps")
            for j in range(4):
                k = h * 4 + j
                nc.tensor.transpose(
                    pt[:, j * 128:(j + 1) * 128],
                    src_tile[:, k * 128:(k + 1) * 128],
                    tident,
                )
            pts.append(pt)
        return pts

    # ---------------- stage B: load x, transpose to xT[d, k, x] ----------
    # xT[p, k, i*128+j] = x[b, i*128+j, k*128+p]; xTb = bf16 shadow for the
    # final out-GEMV (so it doesn't WAR-block next batch's xT rebuild).
    xT = p_xT.tile([128, NK, S], F32R, tag="xT")
    xTb = p_xTb.tile([128, NK, S], BF16, tag="xTb")
    for i in range(NM):
        xn = p_xnat.tile([128, D], tdt, tag="xnat")
        nc.sync.dma_start(xn[:], xin[b, i * 128:(i + 1) * 128, :].bitcast(tdt))
        pts = transpose_8(xn)
        for h in range(2):
            dst = xT[:, h * 4:(h + 1) * 4, i * 128:(i + 1) * 128]
            src = pts[h][:].bitcast(F32).rearrange("p (a c) -> p a c", a=4)
            if h == 0:
                nc.scalar.activation(dst, src, AF.Copy)
            else:
                nc.vector.tensor_copy(dst, src)
        if i == 3 and pending_tail is not None:
            pending_tail()

    # ---------------- stage C: scores + softmax + colsum ------------------
    ybar = p_small.tile([128, NK], F32, tag="ybar")
    psum_c = pc_pool.tile([128, 4 * 512], F32, tag="pc")

    def y_transp(m):
        """load y row-tile m, transpose into yt[p, k*128+j] = y[m*128+j, k*128+p]"""
        yn = p_ynat.tile([128, D], tdt, tag="ynat")
        nc.sync.dma_start(yn[:], yin[b, m * 128:(m + 1) * 128, :].bitcast(tdt))
        yt = p_yTm.tile([128, D], F32R, tag="yTm")
        pts = transpose_8(yn)
        for h in range(2):
            src = pts[h][:].bitcast(F32)
            if h == 0:
                nc.scalar.activation(yt[:, h * 512:(h + 1) * 512], src, AF.Copy)
            else:
                nc.vector.tensor_copy(yt[:, h * 512:(h + 1) * 512], src)
        return yt

    def scores_chunk(yt, Em, rsum, n):
        ps = ps_pool.tile([128, 512], F32, tag="ps")
        for k in range(NK):
            nc.tensor.matmul(
                ps[:],
                lhsT=yt[:, k * 128:(k + 1) * 128],
                rhs=xT[:, k, n * 512:(n + 1) * 512],
                start=(k == 0),
                stop=(k == NK - 1),
            )
        nc.scalar.activation(
            Em[:, n * 512:(n + 1) * 512], ps[:], AF.Exp,
            scale=SCALE, accum_out=rsum[:, n:n + 1],
        )

    def c_mms(m, Em, recip):
        for n in range(NCH):
            nc.tensor.matmul(
                psum_c[0:1, n * 512:(n + 1) * 512],
                lhsT=recip[:],
                rhs=Em[:, n * 512:(n + 1) * 512],
                start=(m == 0),
                stop=(m == NM - 1),
            )

    yt = y_transp(0)
    prev = None  # (m, Em, recip) pending softmax-colsum matmuls
    for m in range(NM):
        # ybar accumulation on DVE (sum of y rows, in transposed layout)
        ytv = yt[:].bitcast(F32).rearrange("p (k j) -> p k j", k=NK)
        if m == 0:
            nc.vector.reduce_sum(ybar[:], ytv, axis=AX.X)
        else:
            ybs = p_small.tile([128, NK], F32, tag="ybs")
            nc.vector.reduce_sum(ybs[:], ytv, axis=AX.X)
            nc.vector.tensor_add(ybar[:], ybar[:], ybs[:])

        if m < NK:
            if m % 2 == 0:
                nc.scalar.activation(xTb[:, m, :], xT[:, m, :].bitcast(F32), AF.Copy)
            else:
                nc.vector.tensor_copy(xTb[:, m, :], xT[:, m, :].bitcast(F32))
        Em = p_E.tile([128, S], F32R, tag="E")
        rsum = p_small.tile([128, NCH], F32, tag="rsum")
        scores_chunk(yt, Em, rsum, 0)
        scores_chunk(yt, Em, rsum, 1)
        yt_next = y_transp(m + 1) if m + 1 < NM else None
        if prev is not None:
            c_mms(*prev)
        scores_chunk(yt, Em, rsum, 2)
        scores_chunk(yt, Em, rsum, 3)
        # rowsum total + reciprocal (DVE)
        rtot = p_small.tile([128, 1], F32, tag="rtot")
        nc.vector.reduce_sum(rtot[:], rsum[:], axis=AX.X)
        recip = p_small.tile([128, 1], F32R, tag="recip")
        with nc.allow_low_precision(reason="softmax recip rounded to f32r for PE"):
            nc.vector.reciprocal(recip[:], rtot[:])
        prev = (m, Em, recip)
        yt = yt_next
    c_mms(*prev)

    # ---------------- stage D: outputs ------------------------------------
    # visual = ybar_scaled @ xT  (PE GEMV, accumulate over k) — independent
    # of the softmax tail, emit first so PE flows straight into it.
    ybar_r = p_small.tile([128, NK], F32R, tag="ybar_r")
    nc.vector.tensor_scalar_mul(ybar_r[:], ybar[:], SCALE / S)
    vis_sb = p_small.tile([1, S], F32, tag="vis_sb")
    for n in range(NCH):
        pv = ps_pool.tile([128, 512], F32, tag="ps")
        for k in range(NK):
            nc.tensor.matmul(
                pv[0:1, :],
                lhsT=ybar_r[:, k:k + 1],
                rhs=xT[:, k, n * 512:(n + 1) * 512],
                start=(k == 0),
                stop=(k == NK - 1),
            )
        nc.scalar.activation(vis_sb[0:1, n * 512:(n + 1) * 512], pv[0:1, :], AF.Copy)
    nc.sync.dma_start(visual[b:b + 1, :], vis_sb[0:1, :])

    # cbar = colmean of softmax weights, bf16, broadcast to all partitions
    csb = p_cb.tile([1, S], BF16, tag="csb")
    with nc.allow_low_precision(reason="cbar rounded to bf16 for the out GEMV"):
        nc.scalar.activation(csb[0:1, :], psum_c[0:1, :], AF.Copy, scale=1.0 / S)
    cbb = p_cb.tile([128, S], BF16, tag="cbb")
    nc.gpsimd.partition_broadcast(cbb[:], csb[0:1, :])

    # out[d] = sum_x cbar[x] * x[x, d] via DVE multiply+accumulate over the
    # bf16 shadow; writes in-place into xTb (dead after this).
    aout_sb = p_small.tile([128, NK], F32, tag="aout_sb")
    for k in range(NK):
        nc.vector.scalar_tensor_tensor(
            out=xTb[:, k, :],
            in0=xTb[:, k, :],
            scalar=1.0,
            in1=cbb[:],
            op0=ALU.mult,
            op1=ALU.mult,
            accum_out=aout_sb[:, k:k + 1],
        )

    def tail():
        # aout [128, NK] -> transpose to [NK, 128] so DRAM write is contiguous
        ptt = ps_pool.tile([128, 512], F32, tag="ps")
        nc.tensor.transpose(ptt[0:NK, 0:128], aout_sb[:], ident[:])
        aout_t = p_small.tile([NK, 128], F32, tag="aout_t")
        nc.scalar.activation(aout_t[0:NK, :], ptt[0:NK, 0:128], AF.Copy)
        nc.sync.dma_start(
            aout[b:b + 1, :].rearrange("o (a c) -> (o a) c", a=NK),
            aout_t[0:NK, :],
        )
    return tail


def build():
    nc = bacc.Bacc("TRN2", target_bir_lowering=False, debug=False,
                   num_devices=NCORES)
    xin = nc.dram_tensor("xin", [PB, S, D], F32, kind="ExternalInput").ap()
    yin = nc.dram_tensor("yin", [PB, S, D], F32, kind="ExternalInput").ap()
    visual = nc.dram_tensor("visual", [PB, S], F32, kind="ExternalOutput").ap()
    aout = nc.dram_tensor("aout", [PB, D], F32, kind="ExternalOutput").ap()
    aps = (xin, yin, visual, aout)

    with contextlib.ExitStack() as ctx:
        tc = ctx.enter_context(tile.TileContext(nc))
        p_xT = ctx.enter_context(tc.tile_pool(name="xT", bufs=1))
        p_xTb = ctx.enter_context(tc.tile_pool(name="xTb", bufs=1))
        p_xnat = ctx.enter_context(tc.tile_pool(name="xnat", bufs=10))
        p_ynat = ctx.enter_context(tc.tile_pool(name="ynat", bufs=3))
        p_yTm = ctx.enter_context(tc.tile_pool(name="yTm", bufs=2))
        p_E = ctx.enter_context(tc.tile_pool(name="E", bufs=2))
        p_cb = ctx.enter_context(tc.tile_pool(name="cb", bufs=1))
        p_small = ctx.enter_context(tc.tile_pool(name="small", bufs=2))
        p_const = ctx.enter_context(tc.tile_pool(name="const", bufs=1))
        ps_pool = ctx.enter_context(tc.tile_pool(name="ps", bufs=4, space="PSUM"))
        pc_pool = ctx.enter_context(tc.tile_pool(name="pc", bufs=1, space="PSUM"))
        pools = (p_xT, p_xTb, p_xnat, p_ynat, p_yTm, p_E, p_cb, p_small,
                 ps_pool, pc_pool)

        ident = p_const.tile([128, 128], F32, tag="ident")
        make_identity(nc, ident[:])
        identr_t = p_const.tile([128, 128], F32R, tag="identr")
        nc.scalar.activation(identr_t[:], ident[:], AF.Copy)
        identr = identr_t[:]

        tail = None
        for b in range(PB):
            tail = _emit_batch(nc, tc, b, aps, pools, ident, identr, tail)
        tail()

    nc.compile()
    nc.m = get_hw_module(nc.m)
    return nc


_NC_CACHE = None


def _get_nc():
    global _NC_CACHE
    if _NC_CACHE is None:
        _NC_CACHE = build()
    return _NC_CACHE


def _run(input_x, input_y, trace=False, **kw):
    nc = _get_nc()
    input_x = np.ascontiguousarray(np.asarray(input_x, dtype=np.float32))
    input_y = np.ascontiguousarray(np.asarray(input_y, dtype=np.float32))
    in_maps = [
        {"xin": input_x[c * PB:(c + 1) * PB], "yin": input_y[c * PB:(c + 1) * PB]}
        for c in range(NCORES)
    ]
    res = run_bass_kernel_spmd(nc, in_maps, core_ids=list(range(NCORES)),
                               trace=trace, **kw)
    visual = np.concatenate([res.results[c]["visual"] for c in range(NCORES)], axis=0)
    aout = np.concatenate([res.results[c]["aout"] for c in range(NCORES)], axis=0)
    return (visual, aout), res


def kernel(input_x, input_y):
    (visual, aout), _ = _run(input_x, input_y)
    return visual, aout


# revision 9
# speedup vs baseline: 1.1592x; 1.0013x over previous
"""Trainium2 Bass kernel for nn_AttentionLayer (B=32, Sx=Sy=2048, D=1024).

reference:
    S   = einsum('byd,bxd->byx', y, x) / sqrt(D)       # [B, Sy, Sx]
    W   = softmax(S, axis=2)
    visual = mean(S, axis=1)                           # [B, Sx]
    out    = mean(W @ x, axis=1)                       # [B, D]

Key algebra: both outputs are means over the Sy axis, so
    visual = (mean_y y) @ x^T / sqrt(D)                 (tiny GEMV)
    out    = cbar @ x,  cbar[x] = mean_y W[y, x]        (tiny GEMV)
Only the scores matmul + softmax colsum are heavy. The second big matmul
(W @ x, equal FLOPs to the first) is eliminated entirely.

Sharding: batch dim across 8 cores, 4 batches per core (pure data parallel).
"""

import contextlib
import numpy as np

import concourse.bass as bass
import concourse.bacc as bacc
import concourse.tile as tile
import concourse.mybir as mybir
from concourse.bass_utils import run_bass_kernel_spmd
from concourse.bass_interp import get_hw_module
from concourse.masks import make_identity

B, S, D = 32, 2048, 1024
NCORES = 8
PB = B // NCORES          # batches per core = 4
NM = S // 128             # 16 row tiles (y)
NK = D // 128             # 8 contraction tiles (d)
NCH = S // 512            # 4 column chunks of 512 (x)
SCALE = 1.0 / 32.0        # 1/sqrt(D)

F32 = mybir.dt.float32
F32R = mybir.dt.float32r
BF16 = mybir.dt.bfloat16
AF = mybir.ActivationFunctionType
ALU = mybir.AluOpType
AX = mybir.AxisListType

TRANSPOSE_F32R = True     # PE transposes at 1.5 cyc/row instead of 2.0


def _emit_batch(nc, tc, b, aps, pools, ident, identr, pending_tail):
    """Emit one batch. Returns a closure emitting this batch's deferred
    aout drain (transpose+copy+DMA), to be emitted inside the NEXT batch so
    the PE doesn't stall on the DVE out-GEMV at the batch boundary."""
    xin, yin, visual, aout = aps
    (p_xT, p_xTb, p_xnat, p_ynat, p_yTm, p_E, p_cb, p_small,
     ps_pool, pc_pool) = pools

    tdt = F32R if TRANSPOSE_F32R else F32
    tident = identr if TRANSPOSE_F32R else ident[:]

    def transpose_8(src_tile):
        """8 PE transposes of one [128, 1024] natural tile into 2 psum tiles;
        returns the two [128, 512] psum tiles (k-chunks 0-3 and 4-7)."""
        pts = []
        for h in range(2):
            pt = ps_pool.tile([128, 512], tdt, tag="ps")
            for j in range(4):
                k = h * 4 + j
                nc.tensor.transpose(
                    pt[:, j * 128:(j + 1) * 128],
                    src_tile[:, k * 128:(k + 1) * 128],
                    tident,
                )
            pts.append(pt)
        return pts

    # ---------------- stage B: load x, transpose to xT[d, k, x] ----------
    # xT[p, k, i*128+j] = x[b, i*128+j, k*128+p]; xTb = bf16 shadow for the
    # final out-GEMV (so it doesn't WAR-block next batch's xT rebuild).
    xT = p_xT.tile([128, NK, S], F32R, tag="xT")
    xTb = p_xTb.tile([128, NK, S], BF16, tag="xTb")
    for i in range(NM):
        xn = p_xnat.tile([128, D], tdt, tag="xnat")
        nc.sync.dma_start(xn[:], xin[b, i * 128:(i + 1) * 128, :].bitcast(tdt))
        pts = transpose_8(xn)
        for h in range(2):
            dst = xT[:, h * 4:(h + 1) * 4, i * 128:(i + 1) * 128]
            src = pts[h][:].bitcast(F32).rearrange("p (a c) -> p a c", a=4)
            if h == 0:
                nc.scalar.activation(dst, src, AF.Copy)
            else:
                nc.vector.tensor_copy(dst, src)


    # ---------------- stage C: scores + softmax + colsum ------------------
    ybar = p_small.tile([128, NK], F32, tag="ybar")
    psum_c = pc_pool.tile([128, 4 * 512], F32, tag="pc")

    def y_transp(m):
        """load y row-tile m, transpose into yt[p, k*128+j] = y[m*128+j, k*128+p]"""
        yn = p_ynat.tile([128, D], tdt, tag="ynat")
        nc.sync.dma_start(yn[:], yin[b, m * 128:(m + 1) * 128, :].bitcast(tdt))
        yt = p_yTm.tile([128, D], F32R, tag="yTm")
        pts = transpose_8(yn)
        for h in range(2):
            src = pts[h][:].bitcast(F32)
            if h == 0:
                nc.scalar.activation(yt[:, h * 512:(h + 1) * 512], src, AF.Copy)
            else:
                nc.vector.tensor_copy(yt[:, h * 512:(h + 1) * 512], src)
        return yt

    def scores_chunk(yt, Em, rsum, n):
        ps = ps_pool.tile([128, 512], F32, tag="ps")
        for k in range(NK):
            nc.tensor.matmul(
                ps[:],
                lhsT=yt[:, k * 128:(k + 1) * 128],
                rhs=xT[:, k, n * 512:(n + 1) * 512],
                start=(k == 0),
                stop=(k == NK - 1),
            )
        nc.scalar.activation(
            Em[:, n * 512:(n + 1) * 512], ps[:], AF.Exp,
            scale=SCALE, accum_out=rsum[:, n:n + 1],
        )

    def c_mms(m, Em, recip):
        for n in range(NCH):
            nc.tensor.matmul(
                psum_c[0:1, n * 512:(n + 1) * 512],
                lhsT=recip[:],
                rhs=Em[:, n * 512:(n + 1) * 512],
                start=(m == 0),
                stop=(m == NM - 1),
            )

    yt = y_transp(0)
    prev = None  # (m, Em, recip) pending softmax-colsum matmuls
    for m in range(NM):
        # ybar accumulation on DVE (sum of y rows, in transposed layout)
        ytv = yt[:].bitcast(F32).rearrange("p (k j) -> p k j", k=NK)
        if m == 0:
            nc.vector.reduce_sum(ybar[:], ytv, axis=AX.X)
        else:
            ybs = p_small.tile([128, NK], F32, tag="ybs")
            nc.vector.reduce_sum(ybs[:], ytv, axis=AX.X)
            nc.vector.tensor_add(ybar[:], ybar[:], ybs[:])

        if m == 2 and pending_tail is not None:
            pending_tail()
        if m < NK:
            if m % 2 == 0:
                nc.scalar.activation(xTb[:, m, :], xT[:, m, :].bitcast(F32), AF.Copy)
            else:
                nc.vector.tensor_copy(xTb[:, m, :], xT[:, m, :].bitcast(F32))
        Em = p_E.tile([128, S], F32R, tag="E")
        rsum = p_small.tile([128, NCH], F32, tag="rsum")
        scores_chunk(yt, Em, rsum, 0)
        scores_chunk(yt, Em, rsum, 1)
        yt_next = y_transp(m + 1) if m + 1 < NM else None
        if prev is not None:
            c_mms(*prev)
        scores_chunk(yt, Em, rsum, 2)
        scores_chunk(yt, Em, rsum, 3)
        # rowsum total + reciprocal (DVE)
        rtot = p_small.tile([128, 1], F32, tag="rtot")
        nc.vector.reduce_sum(rtot[:], rsum[:], axis=AX.X)
        recip = p_small.tile([128, 1], F32R, tag="recip")
        with nc.allow_low_precision(reason="softmax recip rounded to f32r for PE"):
            nc.vector.reciprocal(recip[:], rtot[:])
        prev = (m, Em, recip)
        yt = yt_next
    c_mms(*prev)

    # ---------------- stage D: outputs ------------------------------------
    # visual = ybar_scaled @ xT  (PE GEMV, accumulate over k) — independent
    # of the softmax tail, emit first so PE flows straight into it.
    ybar_r = p_small.tile([128, NK], F32R, tag="ybar_r")
    nc.vector.tensor_scalar_mul(ybar_r[:], ybar[:], SCALE / S)
    vis_sb = p_small.tile([1, S], F32, tag="vis_sb")
    for n in range(NCH):
        pv = ps_pool.tile([128, 512], F32, tag="ps")
        for k in range(NK):
            nc.tensor.matmul(
                pv[0:1, :],
                lhsT=ybar_r[:, k:k + 1],
                rhs=xT[:, k, n * 512:(n + 1) * 512],
                start=(k == 0),
                stop=(k == NK - 1),
            )
        nc.scalar.activation(vis_sb[0:1, n * 512:(n + 1) * 512], pv[0:1, :], AF.Copy)
    nc.sync.dma_start(visual[b:b + 1, :], vis_sb[0:1, :])

    # cbar = colmean of softmax weights, bf16, broadcast to all partitions
    csb = p_cb.tile([1, S], BF16, tag="csb")
    with nc.allow_low_precision(reason="cbar rounded to bf16 for the out GEMV"):
        nc.scalar.activation(csb[0:1, :], psum_c[0:1, :], AF.Copy, scale=1.0 / S)
    cbb = p_cb.tile([128, S], BF16, tag="cbb")
    nc.gpsimd.partition_broadcast(cbb[:], csb[0:1, :])

    # out[d] = sum_x cbar[x] * x[x, d] via DVE multiply+accumulate over the
    # bf16 shadow; writes in-place into xTb (dead after this).
    aout_sb = p_small.tile([128, NK], F32, tag="aout_sb")
    for k in range(NK):
        nc.vector.scalar_tensor_tensor(
            out=xTb[:, k, :],
            in0=xTb[:, k, :],
            scalar=1.0,
            in1=cbb[:],
            op0=ALU.mult,
            op1=ALU.mult,
            accum_out=aout_sb[:, k:k + 1],
        )

    def tail():
        # aout [128, NK] -> transpose to [NK, 128] so DRAM write is contiguous
        ptt = ps_pool.tile([128, 512], F32, tag="ps")
        nc.tensor.transpose(ptt[0:NK, 0:128], aout_sb[:], ident[:])
        aout_t = p_small.tile([NK, 128], F32, tag="aout_t")
        nc.scalar.activation(aout_t[0:NK, :], ptt[0:NK, 0:128], AF.Copy)
        nc.sync.dma_start(
            aout[b:b + 1, :].rearrange("o (a c) -> (o a) c", a=NK),
            aout_t[0:NK, :],
        )
    return tail


def build():
    nc = bacc.Bacc("TRN2", target_bir_lowering=False, debug=False,
                   num_devices=NCORES)
    xin = nc.dram_tensor("xin", [PB, S, D], F32, kind="ExternalInput").ap()
    yin = nc.dram_tensor("yin", [PB, S, D], F32, kind="ExternalInput").ap()
    visual = nc.dram_tensor("visual", [PB, S], F32, kind="ExternalOutput").ap()
    aout = nc.dram_tensor("aout", [PB, D], F32, kind="ExternalOutput").ap()
    aps = (xin, yin, visual, aout)

    with contextlib.ExitStack() as ctx:
        tc = ctx.enter_context(tile.TileContext(nc))
        p_xT = ctx.enter_context(tc.tile_pool(name="xT", bufs=1))
        p_xTb = ctx.enter_context(tc.tile_pool(name="xTb", bufs=1))
        p_xnat = ctx.enter_context(tc.tile_pool(name="xnat", bufs=10))
        p_ynat = ctx.enter_context(tc.tile_pool(name="ynat", bufs=3))
        p_yTm = ctx.enter_context(tc.tile_pool(name="yTm", bufs=2))
        p_E = ctx.enter_context(tc.tile_pool(name="E", bufs=2))
        p_cb = ctx.enter_context(tc.tile_pool(name="cb", bufs=1))
        p_small = ctx.enter_context(tc.tile_pool(name="small", bufs=2))
        p_const = ctx.enter_context(tc.tile_pool(name="const", bufs=1))
        ps_pool = ctx.enter_context(tc.tile_pool(name="ps", bufs=4, space="PSUM"))
        pc_pool = ctx.enter_context(tc.tile_pool(name="pc", bufs=1, space="PSUM"))
        pools = (p_xT, p_xTb, p_xnat, p_ynat, p_yTm, p_E, p_cb, p_small,
                 ps_pool, pc_pool)

        ident = p_const.tile([128, 128], F32, tag="ident")
        make_identity(nc, ident[:])
        identr_t = p_const.tile([128, 128], F32R, tag="identr")
        nc.scalar.activation(identr_t[:], ident[:], AF.Copy)
        identr = identr_t[:]

        tail = None
        for b in range(PB):
            tail = _emit_batch(nc, tc, b, aps, pools, ident, identr, tail)
        tail()

    nc.compile()
    nc.m = get_hw_module(nc.m)
    return nc


_NC_CACHE = None


def _get_nc():
    global _NC_CACHE
    if _NC_CACHE is None:
        _NC_CACHE = build()
    return _NC_CACHE


def _run(input_x, input_y, trace=False, **kw):
    nc = _get_nc()
    input_x = np.ascontiguousarray(np.asarray(input_x, dtype=np.float32))
    input_y = np.ascontiguousarray(np.asarray(input_y, dtype=np.float32))
    in_maps = [
        {"xin": input_x[c * PB:(c + 1) * PB], "yin": input_y[c * PB:(c + 1) * PB]}
        for c in range(NCORES)
    ]
    res = run_bass_kernel_spmd(nc, in_maps, core_ids=list(range(NCORES)),
                               trace=trace, **kw)
    visual = np.concatenate([res.results[c]["visual"] for c in range(NCORES)], axis=0)
    aout = np.concatenate([res.results[c]["aout"] for c in range(NCORES)], axis=0)
    return (visual, aout), res


def kernel(input_x, input_y):
    (visual, aout), _ = _run(input_x, input_y)
    return visual, aout


# revision 17
# speedup vs baseline: 1.2328x; 1.0635x over previous
"""Trainium2 Bass kernel for nn_AttentionLayer (B=32, Sx=Sy=2048, D=1024).

reference:
    S   = einsum('byd,bxd->byx', y, x) / sqrt(D)       # [B, Sy, Sx]
    W   = softmax(S, axis=2)
    visual = mean(S, axis=1)                           # [B, Sx]
    out    = mean(W @ x, axis=1)                       # [B, D]

Key algebra: both outputs are means over the Sy axis, so
    visual = (mean_y y) @ x^T / sqrt(D)                 (tiny GEMV)
    out    = cbar @ x,  cbar[x] = mean_y W[y, x]        (tiny GEMV)
Only the scores matmul + softmax colsum are heavy. The second big matmul
(W @ x, equal FLOPs to the first) is eliminated entirely.

Sharding: batch dim across 8 cores, 4 batches per core (pure data parallel).
"""

import contextlib
import numpy as np

import concourse.bass as bass
import concourse.bacc as bacc
import concourse.tile as tile
import concourse.mybir as mybir
from concourse.bass_utils import run_bass_kernel_spmd
from concourse.bass_interp import get_hw_module
from concourse.masks import make_identity

B, S, D = 32, 2048, 1024
NCORES = 8
PB = B // NCORES          # batches per core = 4
NM = S // 128             # 16 row tiles (y)
NK = D // 128             # 8 contraction tiles (d)
NCH = S // 512            # 4 column chunks of 512 (x)
SCALE = 1.0 / 32.0        # 1/sqrt(D)

F32 = mybir.dt.float32
F32R = mybir.dt.float32r
BF16 = mybir.dt.bfloat16
AF = mybir.ActivationFunctionType
ALU = mybir.AluOpType
AX = mybir.AxisListType

TRANSPOSE_F32R = True     # PE transposes at 1.5 cyc/row instead of 2.0


def _emit_batch(nc, tc, b, aps, pools, ident, identr, deferred_mops, pending_tail, is_last):
    """Emit one batch. Returns a closure emitting this batch's deferred
    aout drain (transpose+copy+DMA), to be emitted inside the NEXT batch so
    the PE doesn't stall on the DVE out-GEMV at the batch boundary."""
    xin, yin, visual, aout = aps
    (p_xT, p_xTb, p_xnat, p_ynat, p_yTm, p_E, p_cb, p_small,
     ps_pool, pc_pool) = pools

    tdt = F32R if TRANSPOSE_F32R else F32
    tident = identr if TRANSPOSE_F32R else ident[:]

    def transpose_8(src_tile):
        """8 PE transposes of one [128, 1024] natural tile into 2 psum tiles;
        returns the two [128, 512] psum tiles (k-chunks 0-3 and 4-7)."""
        pts = []
        for h in range(2):
            pt = ps_pool.tile([128, 512], tdt, tag="ps")
            for j in range(4):
                k = h * 4 + j
                nc.tensor.transpose(
                    pt[:, j * 128:(j + 1) * 128],
                    src_tile[:, k * 128:(k + 1) * 128],
                    tident,
                )
            pts.append(pt)
        return pts

    # ---------------- stage B: load x, transpose to xT[d, k, x] ----------
    # xT[p, k, i*128+j] = x[b, i*128+j, k*128+p]; xTb = bf16 shadow for the
    # final out-GEMV (so it doesn't WAR-block next batch's xT rebuild).
    xT = p_xT.tile([128, NK, S], F32R, tag="xT")
    xTb = p_xTb.tile([128, NK, S], BF16, tag="xTb")
    for i in range(NM):
        xn = p_xnat.tile([128, D], tdt, tag="xnat")
        if b == 0 and i < 2:
            # split the cold-start loads so the first transposes begin sooner
            for q in range(4):
                nc.sync.dma_start(
                    xn[:, q * 256:(q + 1) * 256],
                    xin[b, i * 128:(i + 1) * 128,
                        q * 256:(q + 1) * 256].bitcast(tdt))
        else:
            nc.sync.dma_start(xn[:], xin[b, i * 128:(i + 1) * 128, :].bitcast(tdt))
        pts = transpose_8(xn)
        for h in range(2):
            dst = xT[:, h * 4:(h + 1) * 4, i * 128:(i + 1) * 128]
            src = pts[h][:].bitcast(F32).rearrange("p (a c) -> p a c", a=4)
            if h == 0:
                nc.scalar.activation(dst, src, AF.Copy)
            else:
                nc.vector.tensor_copy(dst, src)


    # ---------------- stage C: scores + softmax + colsum ------------------
    ybar = p_small.tile([128, NK], F32, tag="ybar")
    psum_cA = pc_pool.tile([128, 512], F32, tag="pc")
    psum_cB = pc_pool.tile([128, 512], F32, tag="pc")

    def y_transp(m):
        """load y row-tile m, transpose into yt[p, k*128+j] = y[m*128+j, k*128+p]"""
        yn = p_ynat.tile([128, D], tdt, tag="ynat")
        nc.sync.dma_start(yn[:], yin[b, m * 128:(m + 1) * 128, :].bitcast(tdt))
        yt = p_yTm.tile([128, D], F32R, tag="yTm")
        pts = transpose_8(yn)
        for h in range(2):
            src = pts[h][:].bitcast(F32)
            if h == 0:
                nc.scalar.activation(yt[:, h * 512:(h + 1) * 512], src, AF.Copy)
            else:
                nc.vector.tensor_copy(yt[:, h * 512:(h + 1) * 512], src)
        return yt

    def scores_chunk(yt, Em, rsum, n):
        ps = ps_pool.tile([128, 512], F32, tag="ps")
        for k in range(NK):
            nc.tensor.matmul(
                ps[:],
                lhsT=yt[:, k * 128:(k + 1) * 128],
                rhs=xT[:, k, n * 512:(n + 1) * 512],
                start=(k == 0),
                stop=(k == NK - 1),
            )
        nc.scalar.activation(
            Em[:, n * 512:(n + 1) * 512], ps[:], AF.Exp,
            scale=SCALE, accum_out=rsum[:, n:n + 1],
        )

    def c_mms(m, Em, recip):
        for n in range(NCH):
            bank = psum_cA if n < 2 else psum_cB
            grp = n % 2
            kw = dict(tile_position=(0, 32 * grp)) if grp else {}
            nc.tensor.matmul(
                bank[32 * grp:32 * grp + 32, :],
                lhsT=recip[:],
                rhs=Em[:, n * 512:(n + 1) * 512],
                start=(m == 0),
                stop=(m == NM - 1),
                skip_group_check=True,
                **kw,
            )

    yt = y_transp(0)
    prev = None  # (m, Em, recip) pending softmax-colsum matmuls
    for m in range(NM):
        # ybar accumulation on DVE (sum of y rows, in transposed layout)
        ytv = yt[:].bitcast(F32).rearrange("p (k j) -> p k j", k=NK)
        if m == 0:
            nc.vector.reduce_sum(ybar[:], ytv, axis=AX.X)
        else:
            ybs = p_small.tile([128, NK], F32, tag="ybs")
            nc.vector.reduce_sum(ybs[:], ytv, axis=AX.X)
            nc.vector.tensor_add(ybar[:], ybar[:], ybs[:])

        if m < len(deferred_mops):
            deferred_mops[m]()          # prev batch's out-GEMV op k=m (DVE)
        if m == 9 and pending_tail is not None:
            pending_tail()
        if m < NK:
            if m % 2 == 0:
                nc.scalar.activation(xTb[:, m, :], xT[:, m, :].bitcast(F32), AF.Copy)
            else:
                nc.vector.tensor_copy(xTb[:, m, :], xT[:, m, :].bitcast(F32))
        Em = p_E.tile([128, S], BF16, tag="E")
        rsum = p_small.tile([128, NCH], F32, tag="rsum")
        scores_chunk(yt, Em, rsum, 0)
        scores_chunk(yt, Em, rsum, 1)
        yt_next = y_transp(m + 1) if m + 1 < NM else None
        if prev is not None:
            c_mms(*prev)
        scores_chunk(yt, Em, rsum, 2)
        scores_chunk(yt, Em, rsum, 3)
        # rowsum total + reciprocal (DVE)
        rtot = p_small.tile([128, 1], F32, tag="rtot")
        nc.vector.reduce_sum(rtot[:], rsum[:], axis=AX.X)
        nc.vector.tensor_scalar_mul(rtot[:], rtot[:], float(S))
        recip = p_small.tile([128, 32], BF16, tag="recip")
        nc.gpsimd.memset(recip[:], 0.0)
        with nc.allow_low_precision(reason="softmax recip rounded to bf16 for PE"):
            nc.vector.reciprocal(recip[:, 0:1], rtot[:])
        prev = (m, Em, recip)
        yt = yt_next
    c_mms(*prev)

    # ---------------- stage D: outputs ------------------------------------
    # cbar = colmean of softmax weights, bf16, broadcast to all partitions
    csb = p_cb.tile([1, S], BF16, tag="csb")
    for n in range(NCH):
        bank = psum_cA if n < 2 else psum_cB
        grp = n % 2
        nc.vector.tensor_copy(csb[0:1, n * 512:(n + 1) * 512],
                              bank[32 * grp:32 * grp + 1, :])
    cbb = p_cb.tile([128, S], BF16, tag="cbb")
    nc.gpsimd.partition_broadcast(cbb[:], csb[0:1, :])

    # out[d] = sum_x cbar[x] * x[x, d] via DVE multiply+accumulate over the
    # bf16 shadow; writes in-place into xTb (dead after this). Deferred into
    # the next batch's stage C (one op per m) so the boundary DVE burst
    # doesn't starve the next batch's transpose drains.
    aout_sb = p_small.tile([128, NK], F32, tag="aout_sb")

    def stt_op(k):
        def emit():
            nc.vector.scalar_tensor_tensor(
                out=xTb[:, k, :],
                in0=xTb[:, k, :],
                scalar=1.0,
                in1=cbb[:],
                op0=ALU.mult,
                op1=ALU.mult,
                accum_out=aout_sb[:, k:k + 1],
            )
        return emit
    stt_ops = [stt_op(k) for k in range(NK)]
    if is_last:
        for op in stt_ops:
            op()
        stt_ops = []

    # visual = ybar_scaled @ xT  (PE GEMV, accumulate over k)
    ybar_r = p_small.tile([128, NK], F32R, tag="ybar_r")
    nc.vector.tensor_scalar_mul(ybar_r[:], ybar[:], SCALE / S)
    vis_sb = p_small.tile([1, S], F32, tag="vis_sb")
    for n in range(NCH):
        pv = ps_pool.tile([128, 512], F32, tag="ps")
        for k in range(NK):
            nc.tensor.matmul(
                pv[0:1, :],
                lhsT=ybar_r[:, k:k + 1],
                rhs=xT[:, k, n * 512:(n + 1) * 512],
                start=(k == 0),
                stop=(k == NK - 1),
            )
        nc.scalar.activation(vis_sb[0:1, n * 512:(n + 1) * 512], pv[0:1, :], AF.Copy)
    nc.sync.dma_start(visual[b:b + 1, :], vis_sb[0:1, :])

    def tail():
        # aout [128, NK] -> transpose to [NK, 128] so DRAM write is contiguous
        ptt = ps_pool.tile([128, 512], F32, tag="ps")
        nc.tensor.transpose(ptt[0:NK, 0:128], aout_sb[:], ident[:])
        aout_t = p_small.tile([NK, 128], F32, tag="aout_t")
        nc.scalar.activation(aout_t[0:NK, :], ptt[0:NK, 0:128], AF.Copy)
        nc.sync.dma_start(
            aout[b:b + 1, :].rearrange("o (a c) -> (o a) c", a=NK),
            aout_t[0:NK, :],
        )
    return stt_ops, tail


def build():
    nc = bacc.Bacc("TRN2", target_bir_lowering=False, debug=False,
                   num_devices=NCORES)
    xin = nc.dram_tensor("xin", [PB, S, D], F32, kind="ExternalInput").ap()
    yin = nc.dram_tensor("yin", [PB, S, D], F32, kind="ExternalInput").ap()
    visual = nc.dram_tensor("visual", [PB, S], F32, kind="ExternalOutput").ap()
    aout = nc.dram_tensor("aout", [PB, D], F32, kind="ExternalOutput").ap()
    aps = (xin, yin, visual, aout)

    with contextlib.ExitStack() as ctx:
        tc = ctx.enter_context(tile.TileContext(nc))
        p_xT = ctx.enter_context(tc.tile_pool(name="xT", bufs=1))
        p_xTb = ctx.enter_context(tc.tile_pool(name="xTb", bufs=1))
        p_xnat = ctx.enter_context(tc.tile_pool(name="xnat", bufs=12))
        p_ynat = ctx.enter_context(tc.tile_pool(name="ynat", bufs=3))
        p_yTm = ctx.enter_context(tc.tile_pool(name="yTm", bufs=2))
        p_E = ctx.enter_context(tc.tile_pool(name="E", bufs=2))
        p_cb = ctx.enter_context(tc.tile_pool(name="cb", bufs=1))
        p_small = ctx.enter_context(tc.tile_pool(name="small", bufs=2))
        p_const = ctx.enter_context(tc.tile_pool(name="const", bufs=1))
        ps_pool = ctx.enter_context(tc.tile_pool(name="ps", bufs=6, space="PSUM"))
        pc_pool = ctx.enter_context(tc.tile_pool(name="pc", bufs=2, space="PSUM"))
        pools = (p_xT, p_xTb, p_xnat, p_ynat, p_yTm, p_E, p_cb, p_small,
                 ps_pool, pc_pool)

        ident = p_const.tile([128, 128], F32, tag="ident")
        make_identity(nc, ident[:])
        identr_t = p_const.tile([128, 128], F32R, tag="identr")
        nc.scalar.activation(identr_t[:], ident[:], AF.Copy)
        identr = identr_t[:]

        deferred, tail = [], None
        for b in range(PB):
            deferred, tail = _emit_batch(nc, tc, b, aps, pools, ident, identr,
                                         deferred, tail, b == PB - 1)
        tail()

    nc.compile()
    nc.m = get_hw_module(nc.m)
    return nc


_NC_CACHE = None


def _get_nc():
    global _NC_CACHE
    if _NC_CACHE is None:
        _NC_CACHE = build()
    return _NC_CACHE


def _run(input_x, input_y, trace=False, **kw):
    nc = _get_nc()
    input_x = np.ascontiguousarray(np.asarray(input_x, dtype=np.float32))
    input_y = np.ascontiguousarray(np.asarray(input_y, dtype=np.float32))
    in_maps = [
        {"xin": input_x[c * PB:(c + 1) * PB], "yin": input_y[c * PB:(c + 1) * PB]}
        for c in range(NCORES)
    ]
    res = run_bass_kernel_spmd(nc, in_maps, core_ids=list(range(NCORES)),
                               trace=trace, **kw)
    visual = np.concatenate([res.results[c]["visual"] for c in range(NCORES)], axis=0)
    aout = np.concatenate([res.results[c]["aout"] for c in range(NCORES)], axis=0)
    return (visual, aout), res


def kernel(input_x, input_y):
    (visual, aout), _ = _run(input_x, input_y)
    return visual, aout
